# revision 12
# baseline (speedup 1.0000x reference)
"""NT-Xent instance loss (nn_InstanceLoss) on 8 Trainium2 NeuronCores.

Symmetric ("compute each sim block once") sharding. z = concat(z_i, z_j)
has N=16384 rows, split into 16 kilobands of 1024 rows. Core c owns the
rows of kilobands c (band A) and c+8 (band B). Writing D = (col_kb -
row_kb) mod 16, core c computes sim blocks at distances D=0..8 for band
A and D=0..7 for band B: 17 kiloblocks of [1024, 1024] per core, i.e.
each off-diagonal block of the symmetric sim matrix is computed exactly
once somewhere, and every core does the same amount of work.

For each computed [128 x 512] tile of exp(sim - C_band):
  - row sums (contributions to the block's own rows) come from the
    scalar engine's activation accumulator, and
  - column sums (contributions to the transposed block's rows, by
    symmetry) come from a ones-vector matmul on the tensor engine,
    accumulated across the band's 8 row-tiles into one PSUM bank that
    holds 3 x 512 column-sum slots at partitions 0/32/64 (tile_position
    column-group trick).
The host combines row sums, column sums (un-offset into absolute fp64),
and positives: lse = log(s), loss = mean(lse - pos). D=0 column sums
are discarded (their contributions are already in the row sums).

exp offsets C_band are per-kiloband constants estimated on the host
from ||z_r|| (extreme-value statistics of gaussian dot products);
margins of ~±70 in the exponent make fp32/bf16 over/underflow
impossible for randn-like inputs, and a host-side exact fallback
covers any row that still comes back non-finite.

Per-core uniformity trick: each core receives z rolled by -c*1024 rows
(pre-transposed / sqrt(2)-scaled / bf16-cast on the host), so the same
SPMD program works on every core: band A = local rows 0..1024 vs local
cols 0..9216, band B = local rows 8192..9216 vs local cols 8192..16384,
self-diagonals at local col == local row, positives at local col ==
local row + 8192 (extracted from band A's D=8 block only, which by
symmetry also provides the positives of band B's rows).
"""

import math

import numpy as np
import ml_dtypes

TRAIN_NUM = 8192
EMBED = 128
N = 2 * TRAIN_NUM            # 16384
NCORES = 8
KB = 1024                    # kiloband height
M_PER_BAND = KB // 128       # 8 m-tiles per band
CHUNK = 1536                 # PSUM sim chunk (3 banks); 3 col-sum slots
NEG_BIG = -1.0e30
# E[max_j u.w_j] for ~16k gaussian w_j with per-coord var 2
ROWMAX_COEF = math.sqrt(2.0 * math.log(N - 1)) * math.sqrt(2.0)

# exp2 bit-trick constants for the DVE-offloaded exp tiles:
#   u16 = sat_round(x*A + (BF16_ONE + DELTA - C*A)); bitcast<bf16>(u16) ~ e^(x-C)
# A maps nats to bf16-exponent ulps; DELTA centers the linear-mantissa
# approximation so E[approx/exact] = 1 (the raw trick is biased by
# integral_0^1 (1+t)2^-t dt = 0.5/ln(2)^2 ~ 1.0407).
EXP2_A = 128.0 * math.log2(math.e)
BF16_ONE = 16256.0
EXP2_DELTA = -128.0 * math.log2(0.5 / (math.log(2.0) ** 2))
# m-tiles whose exp runs on the vector engine (rest on scalar/ACT)
DVE_M_FULL = (2, 5, 7)
DVE_M_HALF = (1, 3, 5, 7)

# (band, chunk) list: band A covers local cols 0..9216 (6 full chunks),
# band B covers local cols 8192..16384 (5 full chunks + one 512 chunk).
BANDS = [
    # (band_idx, row_base, col_base, n_chunks, widths)
    (0, 0, 0, [CHUNK] * 6),
    (1, 8192, 8192, [CHUNK] * 5 + [512]),
]
N_COLCHUNKS = 12             # 6 + 6 col-sum tiles DMA'd out

_cached = None


def _build():
    import concourse.bacc as bacc
    import concourse.tile as tile
    from concourse import mybir

    nc = bacc.Bacc(
        "TRN2",
        target_bir_lowering=False,
        debug=False,
        num_devices=NCORES,
    )
    f32 = mybir.dt.float32
    bf16 = mybir.dt.bfloat16

    u16 = mybir.dt.uint16
    zT_dram = nc.dram_tensor("zT", (EMBED, N), bf16, kind="ExternalInput")
    cneg_dram = nc.dram_tensor("c_neg", (128, 4), f32, kind="ExternalInput")
    s_dram = nc.dram_tensor("s_out", (128, 16), f32, kind="ExternalOutput")
    pos_dram = nc.dram_tensor("pos_out", (128, M_PER_BAND), f32, kind="ExternalOutput")
    col_dram = nc.dram_tensor(
        "col_out", (N_COLCHUNKS, 3, 512), f32, kind="ExternalOutput"
    )

    neg_np = np.zeros((128, 128), dtype=np.float32)
    np.fill_diagonal(neg_np, NEG_BIG)
    id_np = np.eye(128, dtype=np.float32)
    neg_dram = nc.inline_tensor(neg_np, name="neg_mask")
    id_dram = nc.inline_tensor(id_np, name="id_mask")
    ones_dram = nc.inline_tensor(
        np.ones((128, 1), dtype=ml_dtypes.bfloat16), name="ones_vec"
    )

    with tile.TileContext(nc) as tc:
        with (
            tc.tile_pool(name="zbuf", bufs=1) as zpool,
            tc.tile_pool(name="consts", bufs=1) as cpool,
            tc.tile_pool(name="persist", bufs=1) as perpool,
            tc.tile_pool(name="psum", bufs=2, space="PSUM") as ppool,
            tc.tile_pool(name="colsum", bufs=2, space="PSUM") as cspool,
            tc.tile_pool(name="expout", bufs=3) as epool,
            tc.tile_pool(name="expu16", bufs=3) as upool,
            tc.tile_pool(name="stage", bufs=2) as stpool,
            tc.tile_pool(name="tmp", bufs=2) as tpool,
        ):
            # small consts first: the diag mask + exp bias gate the first chunk
            negm = cpool.tile([128, 128], f32)
            nc.sync.dma_start(out=negm, in_=neg_dram[:, :])
            idm = cpool.tile([128, 128], f32)
            nc.sync.dma_start(out=idm, in_=id_dram[:, :])
            ones_sb = cpool.tile([128, 1], bf16)
            nc.sync.dma_start(out=ones_sb, in_=ones_dram[:, :])
            cneg_sb = cpool.tile([128, 4], f32)
            nc.sync.dma_start(out=cneg_sb, in_=cneg_dram[:, :])

            # trigger the exp ACT-table load early so it overlaps the z DMA
            # instead of stalling the first real exp
            warm = cpool.tile([128, 1], f32)
            nc.scalar.activation(
                out=warm,
                in_=cneg_sb[:, 0:1],
                func=mybir.ActivationFunctionType.Exp,
                bias=cneg_sb[:, 0:1],
                scale=0.0,
            )

            # z: first the columns the first chunk needs, then the rest
            z_sb = zpool.tile([EMBED, N], bf16)
            cuts = [0, CHUNK, 6144, 10752, N]
            for qs, qe in zip(cuts, cuts[1:]):
                nc.sync.dma_start(
                    out=z_sb[:, qs:qe],
                    in_=zT_dram[:, qs:qe],
                )

            # row-sum accumulator: [band*8+m, chunk] laid out as [128,16,6]
            s_band = perpool.tile([128, 16, 6], f32)
            s_out_sb = perpool.tile([128, 16], f32)
            pos_sb = perpool.tile([128, M_PER_BAND], f32)

            col_k = 0
            for band, row_base, col_base, widths in BANDS:
                for j, width in enumerate(widths):
                    nslots = width // 512
                    cbase = col_base + j * CHUNK
                    colsum = cspool.tile([128, 512], f32)
                    for m in range(M_PER_BAND):
                        r0 = row_base + m * 128
                        ps = ppool.tile([128, CHUNK], f32, tag="ps")
                        for k in range(nslots):
                            nc.tensor.matmul(
                                ps[:, k * 512 : (k + 1) * 512],
                                lhsT=z_sb[:, r0 : r0 + 128],
                                rhs=z_sb[:, cbase + k * 512 : cbase + (k + 1) * 512],
                                start=True,
                                stop=True,
                            )
                        if j == 0:
                            # self-similarity diag at chunk offset 128*m
                            nc.vector.tensor_add(
                                ps[:, m * 128 : m * 128 + 128],
                                ps[:, m * 128 : m * 128 + 128],
                                negm,
                            )
                        if band == 0 and j == 5:
                            # positives at chunk offset 512 + 128*m
                            off = 512 + m * 128
                            tmp = tpool.tile([128, 128], f32)
                            nc.vector.tensor_mul(
                                tmp, ps[:, off : off + 128], idm
                            )
                            nc.vector.tensor_reduce(
                                out=pos_sb[:, m : m + 1],
                                in_=tmp,
                                axis=mybir.AxisListType.X,
                                op=mybir.AluOpType.add,
                            )
                        dve_m = DVE_M_FULL if width == CHUNK else DVE_M_HALF
                        if m in dve_m:
                            ut = upool.tile([128, CHUNK], u16, tag="expu")
                            nc.vector.tensor_scalar(
                                out=ut[:, :width],
                                in0=ps[:, :width],
                                scalar1=EXP2_A,
                                scalar2=cneg_sb[:, 2 + band : 3 + band],
                                op0=mybir.AluOpType.mult,
                                op1=mybir.AluOpType.add,
                            )
                            ev = ut.bitcast(bf16)
                            nc.vector.tensor_reduce(
                                out=s_band[:, band * 8 + m, j : j + 1],
                                in_=ev[:, :width],
                                axis=mybir.AxisListType.X,
                                op=mybir.AluOpType.add,
                            )
                        else:
                            et = epool.tile([128, CHUNK], bf16, tag="exp")
                            nc.scalar.activation(
                                out=et[:, :width],
                                in_=ps[:, :width],
                                func=mybir.ActivationFunctionType.Exp,
                                bias=cneg_sb[:, band : band + 1],
                                scale=1.0,
                                accum_out=s_band[:, band * 8 + m, j : j + 1],
                            )
                            ev = et
                        for s in range(nslots):
                            nc.tensor.matmul(
                                colsum[32 * s : 32 * s + 1, :],
                                lhsT=ones_sb,
                                rhs=ev[:, s * 512 : (s + 1) * 512],
                                start=(m == 0),
                                stop=(m == M_PER_BAND - 1),
                                tile_position=(0, 32 * s),
                            )
                    stage = stpool.tile([128, 512], f32)
                    nc.vector.tensor_copy(stage[0:65, :], colsum[0:65, :])
                    nc.sync.dma_start(
                        out=col_dram[col_k, :, :], in_=stage[0:65:32, :]
                    )
                    col_k += 1

            nc.vector.tensor_reduce(
                out=s_out_sb,
                in_=s_band,
                axis=mybir.AxisListType.X,
                op=mybir.AluOpType.add,
            )
            nc.sync.dma_start(out=s_dram[:, :], in_=s_out_sb)
            nc.sync.dma_start(out=pos_dram[:, :], in_=pos_sb)

    nc.compile()
    return nc


def _get_nc():
    global _cached
    if _cached is None:
        _cached = _build()
    return _cached


def _prep(z_i: np.ndarray, z_j: np.ndarray):
    z = np.concatenate(
        [np.asarray(z_i, np.float32), np.asarray(z_j, np.float32)], axis=0
    )
    w = z * np.float32(math.sqrt(2.0))  # fold 1/T=2 into both operands
    wnorm = np.linalg.norm(w.astype(np.float64), axis=1)
    # per-kiloband exp offset from extreme-value estimate of the row max
    c_band = np.array(
        [
            ROWMAX_COEF * np.median(wnorm[b * KB : (b + 1) * KB])
            for b in range(16)
        ],
        dtype=np.float64,
    )
    in_maps = []
    for c in range(NCORES):
        wc = np.roll(w, -c * KB, axis=0)
        zT = np.ascontiguousarray(wc.T).astype(ml_dtypes.bfloat16)
        cneg = np.zeros((128, 4), dtype=np.float32)
        cneg[:, 0] = -c_band[c]
        cneg[:, 1] = -c_band[c + 8]
        cneg[:, 2] = BF16_ONE + EXP2_DELTA - c_band[c] * EXP2_A
        cneg[:, 3] = BF16_ONE + EXP2_DELTA - c_band[c + 8] * EXP2_A
        in_maps.append({"zT": zT, "c_neg": cneg})
    return w, c_band, in_maps


def _finish(w, c_band, results):
    s_abs = np.zeros(N, dtype=np.float64)
    pos = np.zeros(N, dtype=np.float64)
    for c in range(NCORES):
        r = results[c]
        s_dev = r["s_out"].astype(np.float64)      # [128, 16]
        pos_dev = r["pos_out"].astype(np.float64)  # [128, 8]
        col_dev = r["col_out"].astype(np.float64)  # [12, 3, 512]
        for band, kb in ((0, c), (1, c + 8)):
            scale = math.exp(c_band[kb])
            # row sums: s_dev[p, band*8+m] -> band row m*128+p
            rows = s_dev[:, band * 8 : band * 8 + 8].T.reshape(KB)  # [8*128]
            g0 = kb * KB
            s_abs[g0 : g0 + KB] += rows * scale
            # column sums from this band's 6 chunks
            for j in range(6):
                width = CHUNK if not (band == 1 and j == 5) else 512
                for sl in range(width // 512):
                    L = band * 8192 + j * CHUNK + sl * 512
                    if L < band * 8192 + KB:
                        continue  # D=0 block: already in row sums
                    vals = col_dev[band * 6 + j, sl, :] * scale
                    g = (c * KB + L) % N
                    s_abs[g : g + 512] += vals
        # positives: band A rows and their +8192 partners
        p_rows = pos_dev.T.reshape(KB)
        pos[c * KB : c * KB + KB] = p_rows
        pos[c * KB + 8192 : c * KB + 8192 + KB] = p_rows

    with np.errstate(divide="ignore", invalid="ignore"):
        lse = np.log(s_abs)
    bad = ~np.isfinite(lse)
    if bad.any():
        idx = np.nonzero(bad)[0]
        wb = w[idx].astype(np.float64)
        sim_b = wb @ w.astype(np.float64).T
        for ii, rr in enumerate(idx):
            sim_b[ii, rr] = -np.inf
        m_b = sim_b.max(axis=1)
        lse[idx] = np.log(np.exp(sim_b - m_b[:, None]).sum(axis=1)) + m_b
        pos_idx = np.where(idx < TRAIN_NUM, idx + TRAIN_NUM, idx - TRAIN_NUM)
        pos[idx] = np.einsum("ij,ij->i", wb, w[pos_idx].astype(np.float64))
    loss = (lse - pos).mean()
    return np.float32(loss)


def run(z_i, z_j, trace=False, **kw):
    from concourse.bass_utils import run_bass_kernel_spmd

    nc = _get_nc()
    w, c_band, in_maps = _prep(z_i, z_j)
    res = run_bass_kernel_spmd(
        nc, in_maps, core_ids=list(range(NCORES)), trace=trace, **kw
    )
    return _finish(w, c_band, res.results), res


def kernel(z_i, z_j):
    loss, _ = run(z_i, z_j, trace=False)
    return loss



# revision 20
# speedup vs baseline: 1.2325x; 1.2325x over previous
"""NT-Xent instance loss (nn_InstanceLoss) on 8 Trainium2 NeuronCores.

Symmetric ("compute each sim block once") sharding. z = concat(z_i, z_j)
has N=16384 rows, split into 16 kilobands of 1024 rows. Core c owns the
rows of kilobands c (band A) and c+8 (band B). Writing D = (col_kb -
row_kb) mod 16, core c computes sim blocks at distances D=0..8 for band
A and D=0..7 for band B: 17 kiloblocks of [1024, 1024] per core, i.e.
each off-diagonal block of the symmetric sim matrix is computed exactly
once somewhere, and every core does the same amount of work.

For each computed [128 x 512] tile of exp(sim - C_band):
  - row sums (contributions to the block's own rows) come from the
    scalar engine's activation accumulator, and
  - column sums (contributions to the transposed block's rows, by
    symmetry) come from a ones-vector matmul on the tensor engine,
    accumulated across the band's 8 row-tiles into one PSUM bank that
    holds 3 x 512 column-sum slots at partitions 0/32/64 (tile_position
    column-group trick).
The host combines row sums, column sums (un-offset into absolute fp64),
and positives: lse = log(s), loss = mean(lse - pos). D=0 column sums
are discarded (their contributions are already in the row sums).

exp offsets C_band are per-kiloband constants estimated on the host
from ||z_r|| (extreme-value statistics of gaussian dot products);
margins of ~±70 in the exponent make fp32/bf16 over/underflow
impossible for randn-like inputs, and a host-side exact fallback
covers any row that still comes back non-finite.

Per-core uniformity trick: each core receives z rolled by -c*1024 rows
(pre-transposed / sqrt(2)-scaled / bf16-cast on the host), so the same
SPMD program works on every core: band A = local rows 0..1024 vs local
cols 0..9216, band B = local rows 8192..9216 vs local cols 8192..16384,
self-diagonals at local col == local row, positives at local col ==
local row + 8192 (extracted from band A's D=8 block only, which by
symmetry also provides the positives of band B's rows).
"""

import math

import numpy as np
import ml_dtypes

TRAIN_NUM = 8192
EMBED = 128
N = 2 * TRAIN_NUM            # 16384
NCORES = 8
KB = 1024                    # kiloband height
M_PER_BAND = KB // 128       # 8 m-tiles per band
CHUNK = 1536                 # PSUM sim chunk (3 banks); 3 col-sum slots
NEG_BIG = -1.0e30
# E[max_j u.w_j] for ~16k gaussian w_j with per-coord var 2
ROWMAX_COEF = math.sqrt(2.0 * math.log(N - 1)) * math.sqrt(2.0)

# exp2 bit-trick constants for the DVE-offloaded exp tiles:
#   u16 = sat_round(x*A + (BF16_ONE + DELTA - C*A)); bitcast<bf16>(u16) ~ e^(x-C)
# A maps nats to bf16-exponent ulps; DELTA centers the linear-mantissa
# approximation so E[approx/exact] = 1 (the raw trick is biased by
# integral_0^1 (1+t)2^-t dt = 0.5/ln(2)^2 ~ 1.0407).
EXP2_A = 128.0 * math.log2(math.e)
BF16_ONE = 16256.0
EXP2_DELTA = -128.0 * math.log2(0.5 / (math.log(2.0) ** 2))
# m-tiles whose exp runs on the vector engine (rest on scalar/ACT), chosen
# per chunk to balance ACT ~2.1us/tile vs DVE ~3.7us/tile (measured loaded)
DVE_M_3 = (2, 5, 7)
DVE_M_2 = (3, 6)
DVE_M_HALF = (1, 3, 5, 7)

# (band, chunk) list: band A covers local cols 0..9216 (6 full chunks),
# band B covers local cols 8192..16384 (5 full chunks + one 512 chunk).
BANDS = [
    # (band_idx, row_base, col_base, n_chunks, widths)
    (0, 0, 0, [CHUNK] * 6),
    (1, 8192, 8192, [CHUNK] * 5 + [512]),
]
N_COLCHUNKS = 12             # 6 + 6 col-sum tiles DMA'd out

_cached = None


def _build():
    import concourse.bacc as bacc
    import concourse.tile as tile
    from concourse import mybir

    nc = bacc.Bacc(
        "TRN2",
        target_bir_lowering=False,
        debug=False,
        num_devices=NCORES,
    )
    f32 = mybir.dt.float32
    bf16 = mybir.dt.bfloat16

    u16 = mybir.dt.uint16
    zT_dram = nc.dram_tensor("zT", (EMBED, N), bf16, kind="ExternalInput")
    cneg_dram = nc.dram_tensor("c_neg", (128, 4), f32, kind="ExternalInput")
    s_dram = nc.dram_tensor("s_out", (128, 16), f32, kind="ExternalOutput")
    col_dram = nc.dram_tensor(
        "col_out", (N_COLCHUNKS, 3, 512), f32, kind="ExternalOutput"
    )

    neg_np = np.zeros((128, 128), dtype=np.float32)
    np.fill_diagonal(neg_np, NEG_BIG)
    neg_dram = nc.inline_tensor(neg_np, name="neg_mask")
    ones_dram = nc.inline_tensor(
        np.ones((128, 1), dtype=ml_dtypes.bfloat16), name="ones_vec"
    )

    with tile.TileContext(nc) as tc:
        with (
            tc.tile_pool(name="zbuf", bufs=1) as zpool,
            tc.tile_pool(name="consts", bufs=1) as cpool,
            tc.tile_pool(name="persist", bufs=1) as perpool,
            tc.tile_pool(name="psum", bufs=2, space="PSUM") as ppool,
            tc.tile_pool(name="colsum", bufs=2, space="PSUM") as cspool,
            tc.tile_pool(name="expout", bufs=3) as epool,
            tc.tile_pool(name="expu16", bufs=3) as upool,
            tc.tile_pool(name="stage", bufs=2) as stpool,
        ):
            # tiny consts + the first z slice gate the first matmul; the diag
            # mask is only needed a few us later, so it follows the z head
            ones_sb = cpool.tile([128, 1], bf16)
            nc.sync.dma_start(out=ones_sb, in_=ones_dram[:, :])
            cneg_sb = cpool.tile([128, 4], f32)
            nc.sync.dma_start(out=cneg_sb, in_=cneg_dram[:, :])

            z_sb = zpool.tile([EMBED, N], bf16)
            nc.sync.dma_start(out=z_sb[:, 0:640], in_=zT_dram[:, 0:640])

            negm = cpool.tile([128, 128], f32)
            nc.sync.dma_start(out=negm, in_=neg_dram[:, :])

            # trigger the exp ACT-table load early so it overlaps the z DMA
            # instead of stalling the first real exp
            warm = cpool.tile([128, 1], f32)
            nc.scalar.activation(
                out=warm,
                in_=cneg_sb[:, 0:1],
                func=mybir.ActivationFunctionType.Exp,
                bias=cneg_sb[:, 0:1],
                scale=0.0,
            )

            cuts = [640, CHUNK, 4608, 9216, 12800, N]
            for qs, qe in zip(cuts, cuts[1:]):
                nc.sync.dma_start(
                    out=z_sb[:, qs:qe],
                    in_=zT_dram[:, qs:qe],
                )

            # row-sum accumulator: [band*8+m, chunk] laid out as [128,16,6]
            s_band = perpool.tile([128, 16, 6], f32)
            s_out_sb = perpool.tile([128, 16], f32)

            col_k = 0
            for band, row_base, col_base, widths in BANDS:
                for j, width in enumerate(widths):
                    nslots = width // 512
                    cbase = col_base + j * CHUNK
                    colsum = cspool.tile([128, 512], f32)
                    for m in range(M_PER_BAND):
                        r0 = row_base + m * 128
                        ps = ppool.tile([128, CHUNK], f32, tag="ps")
                        for k in range(nslots):
                            nc.tensor.matmul(
                                ps[:, k * 512 : (k + 1) * 512],
                                lhsT=z_sb[:, r0 : r0 + 128],
                                rhs=z_sb[:, cbase + k * 512 : cbase + (k + 1) * 512],
                                start=True,
                                stop=True,
                            )
                        if j == 0:
                            # self-similarity diag at chunk offset 128*m
                            nc.vector.tensor_add(
                                ps[:, m * 128 : m * 128 + 128],
                                ps[:, m * 128 : m * 128 + 128],
                                negm,
                            )
                        if width != CHUNK:
                            dve_m = DVE_M_HALF
                        elif band == 1 and j >= 3:
                            dve_m = DVE_M_2
                        else:
                            dve_m = DVE_M_3
                        if m in dve_m:
                            ut = upool.tile([128, CHUNK], u16, tag="expu")
                            nc.vector.tensor_scalar(
                                out=ut[:, :width],
                                in0=ps[:, :width],
                                scalar1=EXP2_A,
                                scalar2=cneg_sb[:, 2 + band : 3 + band],
                                op0=mybir.AluOpType.mult,
                                op1=mybir.AluOpType.add,
                            )
                            ev = ut.bitcast(bf16)
                            nc.vector.tensor_reduce(
                                out=s_band[:, band * 8 + m, j : j + 1],
                                in_=ev[:, :width],
                                axis=mybir.AxisListType.X,
                                op=mybir.AluOpType.add,
                            )
                        else:
                            et = epool.tile([128, CHUNK], bf16, tag="exp")
                            nc.scalar.activation(
                                out=et[:, :width],
                                in_=ps[:, :width],
                                func=mybir.ActivationFunctionType.Exp,
                                bias=cneg_sb[:, band : band + 1],
                                scale=1.0,
                                accum_out=s_band[:, band * 8 + m, j : j + 1],
                            )
                            ev = et
                        for s in range(nslots):
                            nc.tensor.matmul(
                                colsum[32 * s : 32 * s + 1, :],
                                lhsT=ones_sb,
                                rhs=ev[:, s * 512 : (s + 1) * 512],
                                start=(m == 0),
                                stop=(m == M_PER_BAND - 1),
                                tile_position=(0, 32 * s),
                            )
                    stage = stpool.tile([128, 512], f32)
                    nc.vector.tensor_copy(stage[0:65, :], colsum[0:65, :])
                    nc.sync.dma_start(
                        out=col_dram[col_k, :, :], in_=stage[0:65:32, :]
                    )
                    col_k += 1

            nc.vector.tensor_reduce(
                out=s_out_sb,
                in_=s_band,
                axis=mybir.AxisListType.X,
                op=mybir.AluOpType.add,
            )
            nc.sync.dma_start(out=s_dram[:, :], in_=s_out_sb)

    nc.compile()
    return nc


def _get_nc():
    global _cached
    if _cached is None:
        _cached = _build()
    return _cached


def _prep(z_i: np.ndarray, z_j: np.ndarray):
    z = np.concatenate(
        [np.asarray(z_i, np.float32), np.asarray(z_j, np.float32)], axis=0
    )
    w = z * np.float32(math.sqrt(2.0))  # fold 1/T=2 into both operands
    wnorm = np.linalg.norm(w.astype(np.float64), axis=1)
    # per-kiloband exp offset from extreme-value estimate of the row max
    c_band = np.array(
        [
            ROWMAX_COEF * np.median(wnorm[b * KB : (b + 1) * KB])
            for b in range(16)
        ],
        dtype=np.float64,
    )
    in_maps = []
    for c in range(NCORES):
        wc = np.roll(w, -c * KB, axis=0)
        zT = np.ascontiguousarray(wc.T).astype(ml_dtypes.bfloat16)
        cneg = np.zeros((128, 4), dtype=np.float32)
        cneg[:, 0] = -c_band[c]
        cneg[:, 1] = -c_band[c + 8]
        cneg[:, 2] = BF16_ONE + EXP2_DELTA - c_band[c] * EXP2_A
        cneg[:, 3] = BF16_ONE + EXP2_DELTA - c_band[c + 8] * EXP2_A
        in_maps.append({"zT": zT, "c_neg": cneg})
    return w, c_band, in_maps


def _finish(w, c_band, results):
    s_abs = np.zeros(N, dtype=np.float64)
    # positives on the host: O(N*d), negligible next to the device's O(N^2*d)
    w64 = w.astype(np.float64)
    pos = (w64 * np.roll(w64, -TRAIN_NUM, axis=0)).sum(axis=1)
    for c in range(NCORES):
        r = results[c]
        s_dev = r["s_out"].astype(np.float64)      # [128, 16]
        col_dev = r["col_out"].astype(np.float64)  # [12, 3, 512]
        for band, kb in ((0, c), (1, c + 8)):
            scale = math.exp(c_band[kb])
            # row sums: s_dev[p, band*8+m] -> band row m*128+p
            rows = s_dev[:, band * 8 : band * 8 + 8].T.reshape(KB)  # [8*128]
            g0 = kb * KB
            s_abs[g0 : g0 + KB] += rows * scale
            # column sums from this band's 6 chunks
            for j in range(6):
                width = CHUNK if not (band == 1 and j == 5) else 512
                for sl in range(width // 512):
                    L = band * 8192 + j * CHUNK + sl * 512
                    if L < band * 8192 + KB:
                        continue  # D=0 block: already in row sums
                    vals = col_dev[band * 6 + j, sl, :] * scale
                    g = (c * KB + L) % N
                    s_abs[g : g + 512] += vals

    with np.errstate(divide="ignore", invalid="ignore"):
        lse = np.log(s_abs)
    bad = ~np.isfinite(lse)
    if bad.any():
        idx = np.nonzero(bad)[0]
        wb = w[idx].astype(np.float64)
        sim_b = wb @ w.astype(np.float64).T
        for ii, rr in enumerate(idx):
            sim_b[ii, rr] = -np.inf
        m_b = sim_b.max(axis=1)
        lse[idx] = np.log(np.exp(sim_b - m_b[:, None]).sum(axis=1)) + m_b
        pos_idx = np.where(idx < TRAIN_NUM, idx + TRAIN_NUM, idx - TRAIN_NUM)
        pos[idx] = np.einsum("ij,ij->i", wb, w[pos_idx].astype(np.float64))
    loss = (lse - pos).mean()
    return np.float32(loss)


def run(z_i, z_j, trace=False, **kw):
    from concourse.bass_utils import run_bass_kernel_spmd

    nc = _get_nc()
    w, c_band, in_maps = _prep(z_i, z_j)
    res = run_bass_kernel_spmd(
        nc, in_maps, core_ids=list(range(NCORES)), trace=trace, **kw
    )
    return _finish(w, c_band, res.results), res


def kernel(z_i, z_j):
    loss, _ = run(z_i, z_j, trace=False)
    return loss

